# revision 7
# baseline (speedup 1.0000x reference)
"""GAT (2-layer, CITESEER-like) on 8 TRN2 NeuronCores via Bass/Tile.

Sharding: nodes partitioned by destination across 8 cores; within a core,
dst nodes are sorted by in-degree and grouped into blocks of 128. Edges are
bucketed per (core, block) and laid out as [128 dst-slots, C_b chunks] where
slot (i, c) holds the c-th in-edge of block-dst i, so segment-softmax stats
are per-partition-local. Each layer computes its node features + folded
attention projections locally ([W | W@att_src | W@att_dst]), packs them into
512B bf16 table rows, AllGathers the table, then aggregates edges per block:
one indirect row-gather per block, exp(leaky_relu(logit)) on DVE/ACT, and a
PSUM accumulation of p-weighted rows via identity matmuls (denominator rides
along as a folded ones-column). Small weights are replicated.
"""

import numpy as np
import ml_dtypes

import concourse.bass as bass
import concourse.mybir as mybir
import concourse.tile as tile
from concourse.bass import AP, IndirectOffsetOnAxis
from concourse.masks import make_identity

# ---------------------------------------------------------------------------
# walrus in this container accepts only ONE sync wait per instruction, but
# Tile attaches one wait per producer sem lane. Split: insert same-engine
# wait-only nops ahead of any instruction carrying multiple waits.
# ---------------------------------------------------------------------------
def split_multi_waits(nc):
    n = 0
    for bb in nc.main_func.blocks:
        insts = bb.instructions
        out = []
        for ins in insts:
            si = ins.sync_info
            waits = list(si.on_wait) if (si is not None and si.on_wait) else []
            if len(waits) > 1:
                for w in waits[:-1]:
                    nop = mybir.InstNoOp(
                        name=f"wsplit-{n}", ins=[], outs=[],
                        sync_info=mybir.SyncInfo(on_wait=[w], on_update=[]),
                    )
                    n += 1
                    nop.engine = ins.engine
                    nc.register_instruction(nop, overwrite=True)
                    out.append(nop)
                si.on_wait = waits[-1:]
            out.append(ins)
        if len(out) != len(insts):
            insts.clear()
            insts.extend(out)
    return n

# walrus ships with DGE vector-offset lowering off ("DynamicDMA is disabled"),
# which silently turns indirect_dma_start into a sequential copy. Enable it.
import concourse.bass_utils as _bu

if not getattr(_bu, "_gat_dge_patched", False):
    _orig_gwa = _bu.get_walrus_args

    def _gwa(*a, **kw):
        return [
            *_orig_gwa(*a, **kw),
            "--dge-levels=io,scalar_dynamic_offset,vector_dynamic_offsets",
        ]

    _bu.get_walrus_args = _gwa
    _bu._gat_dge_patched = True

F32 = mybir.dt.float32
BF16 = mybir.dt.bfloat16
I32 = mybir.dt.int32
P = 128
NEG_SLOPE = 0.2
MASK_NEG = -1.0e30


def full_cfg():
    return dict(N=50000, E=800000, FIN=3703, HID=100, H1=2, H2=1, C=6, NC=8)


def derive(cfg):
    d = dict(cfg)
    d["D1"] = cfg["H1"] * cfg["HID"]          # layer-1 output width (concat)
    d["ND"] = cfg["N"] // cfg["NC"]           # real dsts per core
    d["NBLK"] = -(-d["ND"] // P)              # blocks per core
    d["NDP"] = d["NBLK"] * P                  # padded dsts per core
    d["KCH1"] = -(-cfg["FIN"] // P)           # k-chunks for x@W1
    d["FINP"] = d["KCH1"] * P
    d["KCH2"] = -(-d["D1"] // P)              # k-chunks for D1 contractions
    d["DX1"] = d["D1"] + 2 * cfg["H1"]        # W1ext cols
    d["DX2"] = d["D1"] + 2 * cfg["H2"]        # W2ext cols
    # table row layout (bf16 slots)
    d["A1OFF"] = cfg["H1"] * (cfg["HID"] + 1)           # l1: per-head [feat|1]
    au1 = d["A1OFF"] + (d["A1OFF"] % 2)
    d["AUX1F32"] = au1 // 2                              # a_src f32 elems
    used1 = au1 + 2 * cfg["H1"]
    d["A2OFF"] = d["D1"] + 1                             # l2: [feat|1]
    au2 = d["A2OFF"] + (d["A2OFF"] % 2)
    d["AUX2F32"] = au2 // 2
    used2 = au2 + 2 * cfg["H2"]
    d["ROW"] = -(-max(used1, used2) // P) * P            # 256B-multiple rows
    d["ROWF32"] = d["ROW"] // 2
    d["NROWS"] = cfg["NC"] * d["NDP"]                    # table rows
    return d


# ---------------------------------------------------------------------------
# Host-side preprocessing (sharding): bucket/sort edges, build index planes.
# ---------------------------------------------------------------------------
def preprocess(cfg, edge_index):
    c = derive(cfg)
    N, NC, ND, NBLK, NDP = cfg["N"], cfg["NC"], c["ND"], c["NBLK"], c["NDP"]

    src = np.asarray(edge_index[0], dtype=np.int64)
    dst = np.asarray(edge_index[1], dtype=np.int64)
    loops = np.arange(N, dtype=np.int64)
    src = np.concatenate([src, loops])
    dst = np.concatenate([dst, loops])

    deg = np.bincount(dst, minlength=N)

    # per-core degree-sorted permutation of the core's dst range
    newpos = np.empty(N, dtype=np.int64)        # old node id -> table row
    order_all = np.empty(N, dtype=np.int64)     # table row (real part) -> old id
    for k in range(NC):
        lo = k * ND
        nodes = np.arange(lo, lo + ND)
        order = nodes[np.argsort(-deg[nodes], kind="stable")]
        newpos[order] = k * NDP + np.arange(ND)
        order_all[lo : lo + ND] = order

    src_n = newpos[src]
    dst_n = newpos[dst]

    # per (core, block, slot) chunk counts
    deg_n = np.zeros(NC * NDP, dtype=np.int64)
    np.add.at(deg_n, dst_n, 1)
    deg_blk = deg_n.reshape(NC, NBLK, P)
    cblk = deg_blk.max(axis=2).max(axis=0)      # shared across cores
    CT = int(cblk.sum())
    off = np.zeros(NBLK, dtype=np.int64)
    off[1:] = np.cumsum(cblk)[:-1]

    # slot grids
    idx = np.zeros((NC, P, CT), dtype=np.int32)
    maskb = np.full((NC, P, CT), MASK_NEG, dtype=np.float32)

    # order edges per dst by src (locality), then place into slots
    eorder = np.lexsort((src_n, dst_n))
    src_s = src_n[eorder]
    dst_s = dst_n[eorder]
    # rank of each edge within its dst run
    starts = np.zeros(NC * NDP + 1, dtype=np.int64)
    np.cumsum(np.bincount(dst_s, minlength=NC * NDP), out=starts[1:])
    rank = np.arange(len(dst_s)) - starts[dst_s]

    core = dst_s // NDP
    slot = dst_s % NDP % P
    blk = (dst_s % NDP) // P
    colpos = off[blk] + rank
    idx[core, slot, colpos] = src_s.astype(np.int32)
    maskb[core, slot, colpos] = 0.0

    return dict(
        cblk=[int(x) for x in cblk],
        CT=CT,
        off=[int(x) for x in off],
        idx=idx,
        maskb=maskb,
        newpos=newpos,
        order_all=order_all,
    )


def fold_weights(cfg, W1, att_src1, att_dst1, W2, att_src2, att_dst2):
    c = derive(cfg)
    H1, HID, D1 = cfg["H1"], cfg["HID"], c["D1"]
    # a_src[n,h] = sum_f h[n,h,f] att_src[h,f]  ->  x @ wsrc, wsrc[:,h] folded
    W1r = W1.reshape(cfg["FIN"], H1, HID)
    wsrc1 = np.einsum("khf,hf->kh", W1r, att_src1)
    wdst1 = np.einsum("khf,hf->kh", W1r, att_dst1)
    w1ext = np.concatenate([W1, wsrc1, wdst1], axis=1)          # [FIN, DX1]
    w1ext = np.concatenate(
        [w1ext, np.zeros((c["FINP"] - cfg["FIN"], c["DX1"]), w1ext.dtype)], axis=0
    )
    W2r = W2.reshape(D1, cfg["H2"], D1)
    wsrc2 = np.einsum("khf,hf->kh", W2r, att_src2)
    wdst2 = np.einsum("khf,hf->kh", W2r, att_dst2)
    w2ext = np.concatenate([W2, wsrc2, wdst2], axis=1)          # [D1, DX2]
    return w1ext, w2ext


# ---------------------------------------------------------------------------
# Device program (SPMD, one Bass module for all cores)
# ---------------------------------------------------------------------------
def build_program(cfg, cblk, CT, off, debug=False):
    c = derive(cfg)
    NC, HID, H1, H2, CL = cfg["NC"], cfg["HID"], cfg["H1"], cfg["H2"], cfg["C"]
    D1, NBLK, NDP, KCH1, KCH2 = c["D1"], c["NBLK"], c["NDP"], c["KCH1"], c["KCH2"]
    DX1, DX2, ROW, ROWF32, NROWS = c["DX1"], c["DX2"], c["ROW"], c["ROWF32"], c["NROWS"]
    A1OFF, AUX1F32, A2OFF, AUX2F32 = c["A1OFF"], c["AUX1F32"], c["A2OFF"], c["AUX2F32"]
    FINP = c["FINP"]

    nc = bass.Bass()
    xt_in = nc.declare_dram_parameter("xt", [NBLK * P, KCH1 * P], BF16, isOutput=False)
    idx_in = nc.declare_dram_parameter("idx", [P, CT], I32, isOutput=False)
    mask_in = nc.declare_dram_parameter("mask", [P, CT], F32, isOutput=False)
    w1_in = nc.declare_dram_parameter("w1ext", [FINP, DX1], BF16, isOutput=False)
    w2_in = nc.declare_dram_parameter("w2ext", [D1, DX2], BF16, isOutput=False)
    wl_in = nc.declare_dram_parameter("wl", [D1, CL], BF16, isOutput=False)
    b1_in = nc.declare_dram_parameter("b1b", [P, D1], F32, isOutput=False)
    b2_in = nc.declare_dram_parameter("b2b", [P, D1], F32, isOutput=False)
    bl_in = nc.declare_dram_parameter("blb", [P, CL], F32, isOutput=False)
    out_dram = nc.declare_dram_parameter("out", [NDP, CL], F32, isOutput=True)
    if debug:
        d_sh1 = nc.declare_dram_parameter("d_sh1", [NDP, ROW], BF16, isOutput=True)
        d_g0 = nc.declare_dram_parameter("d_g0", [P, cblk[0] * ROW], BF16, isOutput=True)
        d_e0 = nc.declare_dram_parameter("d_e0", [P, cblk[0] * H1], F32, isOutput=True)
        d_ps0 = nc.declare_dram_parameter("d_ps0", [P, H1 * (HID + 1)], F32, isOutput=True)
        d_sh2 = nc.declare_dram_parameter("d_sh2", [NDP, ROW], BF16, isOutput=True)

    shard1 = nc.dram_tensor("shard1", [NDP, ROW], BF16)
    table1 = nc.dram_tensor("table1", [NROWS, ROW], BF16, addr_space="Shared")
    shard2 = nc.dram_tensor("shard2", [NDP, ROW], BF16)
    table2 = nc.dram_tensor("table2", [NROWS, ROW], BF16, addr_space="Shared")
    groups = [list(range(NC))]

    with tile.TileContext(nc) as tc:
        with (
            tc.tile_pool(name="persist", bufs=1) as pp,
            tc.tile_pool(name="work", bufs=4) as wp,
            tc.tile_pool(name="gath", bufs=2) as gp,
            tc.tile_pool(name="xload", bufs=2) as xp,
            tc.tile_pool(name="psacc", bufs=3, space="PSUM") as psa,
            tc.tile_pool(name="pstp", bufs=2, space="PSUM") as pst,
            tc.tile_pool(name="pshd", bufs=2, space="PSUM") as psh,
        ):
            # --- persistent constants ---
            ident = pp.tile([P, P], BF16, tag="ident")
            make_identity(nc, ident[:])
            idxp = pp.tile([P, CT], I32, tag="idxp")
            nc.sync.dma_start(idxp[:], idx_in[:])
            maskp = pp.tile([P, CT], F32, tag="maskp")
            nc.sync.dma_start(maskp[:], mask_in[:])
            w1sb = pp.tile([P, KCH1 * DX1], BF16, tag="w1sb")
            for kc in range(KCH1):
                nc.sync.dma_start(
                    w1sb[:, kc * DX1 : (kc + 1) * DX1],
                    w1_in[kc * P : (kc + 1) * P, :],
                )
            w2sb = pp.tile([P, KCH2 * DX2], BF16, tag="w2sb")
            wlsb = pp.tile([P, KCH2 * CL], BF16, tag="wlsb")
            for kc in range(KCH2):
                ksz = min(P, D1 - kc * P)
                nc.sync.dma_start(
                    w2sb[:ksz, kc * DX2 : kc * DX2 + DX2],
                    w2_in[kc * P : kc * P + ksz, :],
                )
                nc.sync.dma_start(
                    wlsb[:ksz, kc * CL : kc * CL + CL],
                    wl_in[kc * P : kc * P + ksz, :],
                )
            b1sb = pp.tile([P, D1], F32, tag="b1sb")
            nc.sync.dma_start(b1sb[:], b1_in[:])
            b2sb = pp.tile([P, D1], F32, tag="b2sb")
            nc.sync.dma_start(b2sb[:], b2_in[:])
            blsb = pp.tile([P, CL], F32, tag="blsb")
            nc.sync.dma_start(blsb[:], bl_in[:])
            adst1 = pp.tile([P, NBLK * H1], F32, tag="adst1")
            adst2 = pp.tile([P, NBLK * H2], F32, tag="adst2")

            # --- phase A: hext1 = x @ [W1|wsrc1|wdst1], pack table rows ---
            for nb in range(NBLK):
                xsb = xp.tile([P, KCH1 * P], BF16, tag="xsb")
                nc.sync.dma_start(xsb[:], xt_in[nb * P : (nb + 1) * P, :])
                ps = psa.tile([P, DX1], F32, tag="acc")
                for kc in range(KCH1):
                    nc.tensor.matmul(
                        ps[:],
                        lhsT=xsb[:, kc * P : (kc + 1) * P],
                        rhs=w1sb[:, kc * DX1 : (kc + 1) * DX1],
                        start=(kc == 0),
                        stop=(kc == KCH1 - 1),
                    )
                tbl = wp.tile([P, ROW], BF16, tag="tbl")
                nc.gpsimd.memset(tbl[:, A1OFF:ROW], 0.0)
                for h in range(H1):
                    nc.scalar.copy(
                        tbl[:, h * (HID + 1) : h * (HID + 1) + HID],
                        ps[:, h * HID : (h + 1) * HID],
                    )
                    nc.vector.memset(tbl[:, h * (HID + 1) + HID : h * (HID + 1) + HID + 1], 1.0)
                nc.vector.tensor_copy(
                    tbl[:].bitcast(F32)[:, AUX1F32 : AUX1F32 + H1],
                    ps[:, D1 : D1 + H1],
                )
                nc.vector.tensor_copy(
                    adst1[:, nb * H1 : (nb + 1) * H1], ps[:, D1 + H1 : D1 + 2 * H1]
                )
                nc.sync.dma_start(shard1[nb * P : (nb + 1) * P, :], tbl[:])

            nc.gpsimd.collective_compute(
                "AllGather", mybir.AluOpType.bypass, replica_groups=groups,
                ins=[shard1[:]], outs=[table1[:]],
            )
            if debug:
                nc.sync.dma_start(d_sh1[:], shard1[:])

            # --- phase B: layer-1 edge aggregation + layer-2 features ---
            for nb in range(NBLK):
                Cb = cblk[nb]
                ob = off[nb]
                G = gp.tile([P, Cb * ROW], BF16, tag="G")
                for cc in range(Cb):
                    nc.gpsimd.indirect_dma_start(
                        out=G[:, cc * ROW : (cc + 1) * ROW], out_offset=None,
                        in_=table1[:],
                        in_offset=IndirectOffsetOnAxis(
                            ap=idxp[:, ob + cc : ob + cc + 1], axis=0
                        ),
                    )
                Gf = G[:].bitcast(F32)
                asrc = AP(Gf.tensor, Gf.offset, [Gf.ap[0], [ROWF32, Cb], [1, H1]])
                asrc = asrc.add_offset_elems(AUX1F32) if hasattr(asrc, "add_offset_elems") else AP(
                    Gf.tensor, Gf.offset + AUX1F32, [Gf.ap[0], [ROWF32, Cb], [1, H1]]
                )
                ad = adst1[:, nb * H1 : (nb + 1) * H1]
                adb = AP(ad.tensor, ad.offset, [ad.ap[0], [0, Cb], [1, 1]] if H1 == 1 else [ad.ap[0], [0, Cb], [ad.ap[-1][0], H1]])
                mk = maskp[:, ob : ob + Cb]
                mkb = AP(mk.tensor, mk.offset, [mk.ap[0], [mk.ap[-1][0], Cb], [0, H1]])
                e = wp.tile([P, Cb * H1], F32, tag="e")
                nc.vector.tensor_tensor(out=e[:], in0=asrc, in1=adb, op=mybir.AluOpType.add)
                nc.vector.tensor_tensor(out=e[:], in0=e[:], in1=mkb, op=mybir.AluOpType.add)
                lk = wp.tile([P, Cb * H1], F32, tag="lk")
                nc.vector.tensor_scalar_mul(lk[:], e[:], NEG_SLOPE)
                nc.vector.tensor_tensor(out=e[:], in0=e[:], in1=lk[:], op=mybir.AluOpType.max)
                pt = wp.tile([P, Cb * H1], F32, tag="pt")
                nc.scalar.activation(pt[:], e[:], mybir.ActivationFunctionType.Exp)
                if debug and nb == 0:
                    nc.sync.dma_start(d_g0[:], G[:])
                    nc.sync.dma_start(d_e0[:], e[:])

                ps = psa.tile([P, A1OFF], F32, tag="acc")
                for cc in range(Cb):
                    gw = wp.tile([P, A1OFF], BF16, tag="gw")
                    for h in range(H1):
                        nc.vector.tensor_scalar_mul(
                            gw[:, h * (HID + 1) : (h + 1) * (HID + 1)],
                            G[:, cc * ROW + h * (HID + 1) : cc * ROW + (h + 1) * (HID + 1)],
                            pt[:, cc * H1 + h : cc * H1 + h + 1],
                        )
                    nc.tensor.matmul(
                        ps[:], lhsT=ident[:], rhs=gw[:],
                        start=(cc == 0), stop=(cc == Cb - 1),
                    )

                if debug and nb == 0:
                    pscp = wp.tile([P, A1OFF], F32, tag="pscp")
                    nc.vector.tensor_copy(pscp[:], ps[:])
                    nc.sync.dma_start(d_ps0[:], pscp[:])
                # normalize, bias, ELU -> h2 (bf16)
                dn = wp.tile([P, H1], F32, tag="dn")
                dview = AP(ps.tensor, ps[:].offset + HID, [ps[:].ap[0], [HID + 1, H1]])
                nc.vector.tensor_scalar_add(dn[:], dview, 1e-16)
                r = wp.tile([P, H1], F32, tag="r")
                nc.vector.reciprocal(r[:], dn[:])
                o1 = wp.tile([P, D1], F32, tag="o1")
                for h in range(H1):
                    nc.vector.tensor_scalar_mul(
                        o1[:, h * HID : (h + 1) * HID],
                        ps[:, h * (HID + 1) : h * (HID + 1) + HID],
                        r[:, h : h + 1],
                    )
                nc.vector.tensor_tensor(out=o1[:], in0=o1[:], in1=b1sb[:], op=mybir.AluOpType.add)
                tm = wp.tile([P, D1], F32, tag="tm")
                nc.vector.tensor_scalar_min(tm[:], o1[:], 0.0)
                te = wp.tile([P, D1], F32, tag="te")
                nc.scalar.activation(te[:], tm[:], mybir.ActivationFunctionType.Exp)
                tr = wp.tile([P, D1], F32, tag="tr")
                nc.vector.tensor_scalar_max(tr[:], o1[:], 0.0)
                nc.vector.tensor_tensor(out=te[:], in0=te[:], in1=tr[:], op=mybir.AluOpType.add)
                h2b = wp.tile([P, D1], BF16, tag="h2b")
                nc.vector.tensor_scalar_add(h2b[:], te[:], -1.0)

                # hext2 = h2 @ [W2|wsrc2|wdst2] via per-chunk PE transpose
                ps2 = psa.tile([P, DX2], F32, tag="acc")
                for kc in range(KCH2):
                    ksz = min(P, D1 - kc * P)
                    tp = pst.tile([P, P], BF16, tag="tp")
                    nc.tensor.transpose(tp[:ksz, :], h2b[:, kc * P : kc * P + ksz], ident[:])
                    h2t = wp.tile([P, P], BF16, tag="h2t")
                    nc.vector.tensor_copy(h2t[:ksz, :], tp[:ksz, :])
                    nc.tensor.matmul(
                        ps2[:], lhsT=h2t[:ksz, :], rhs=w2sb[:ksz, kc * DX2 : kc * DX2 + DX2],
                        start=(kc == 0), stop=(kc == KCH2 - 1),
                    )
                tbl = wp.tile([P, ROW], BF16, tag="tbl")
                nc.gpsimd.memset(tbl[:, A2OFF:ROW], 0.0)
                nc.scalar.copy(tbl[:, 0:D1], ps2[:, 0:D1])
                nc.vector.memset(tbl[:, D1 : D1 + 1], 1.0)
                nc.vector.tensor_copy(
                    tbl[:].bitcast(F32)[:, AUX2F32 : AUX2F32 + H2],
                    ps2[:, D1 : D1 + H2],
                )
                nc.vector.tensor_copy(
                    adst2[:, nb * H2 : (nb + 1) * H2], ps2[:, D1 + H2 : D1 + 2 * H2]
                )
                nc.sync.dma_start(shard2[nb * P : (nb + 1) * P, :], tbl[:])

            nc.gpsimd.collective_compute(
                "AllGather", mybir.AluOpType.bypass, replica_groups=groups,
                ins=[shard2[:]], outs=[table2[:]],
            )
            if debug:
                nc.sync.dma_start(d_sh2[:], shard2[:])

            # --- phase C: layer-2 edge aggregation + head ---
            for nb in range(NBLK):
                Cb = cblk[nb]
                ob = off[nb]
                G = gp.tile([P, Cb * ROW], BF16, tag="G")
                for cc in range(Cb):
                    nc.gpsimd.indirect_dma_start(
                        out=G[:, cc * ROW : (cc + 1) * ROW], out_offset=None,
                        in_=table2[:],
                        in_offset=IndirectOffsetOnAxis(
                            ap=idxp[:, ob + cc : ob + cc + 1], axis=0
                        ),
                    )
                Gf = G[:].bitcast(F32)
                asrc = AP(Gf.tensor, Gf.offset + AUX2F32, [Gf.ap[0], [ROWF32, Cb], [1, H2]])
                ad = adst2[:, nb * H2 : (nb + 1) * H2]
                adb = AP(ad.tensor, ad.offset, [ad.ap[0], [0, Cb], [1, H2]])
                mk = maskp[:, ob : ob + Cb]
                mkb = AP(mk.tensor, mk.offset, [mk.ap[0], [mk.ap[-1][0], Cb], [0, H2]])
                e = wp.tile([P, Cb * H2], F32, tag="e")
                nc.vector.tensor_tensor(out=e[:], in0=asrc, in1=adb, op=mybir.AluOpType.add)
                nc.vector.tensor_tensor(out=e[:], in0=e[:], in1=mkb, op=mybir.AluOpType.add)
                lk = wp.tile([P, Cb * H2], F32, tag="lk")
                nc.vector.tensor_scalar_mul(lk[:], e[:], NEG_SLOPE)
                nc.vector.tensor_tensor(out=e[:], in0=e[:], in1=lk[:], op=mybir.AluOpType.max)
                pt = wp.tile([P, Cb * H2], F32, tag="pt")
                nc.scalar.activation(pt[:], e[:], mybir.ActivationFunctionType.Exp)

                ps = psa.tile([P, A2OFF], F32, tag="acc")
                for cc in range(Cb):
                    gw = wp.tile([P, A2OFF], BF16, tag="gw")
                    nc.vector.tensor_scalar_mul(
                        gw[:], G[:, cc * ROW : cc * ROW + A2OFF],
                        pt[:, cc * H2 : cc * H2 + 1],
                    )
                    nc.tensor.matmul(
                        ps[:], lhsT=ident[:], rhs=gw[:],
                        start=(cc == 0), stop=(cc == Cb - 1),
                    )

                dn = wp.tile([P, H2], F32, tag="dn")
                nc.vector.tensor_scalar_add(dn[:], ps[:, D1 : D1 + 1], 1e-16)
                r = wp.tile([P, H2], F32, tag="r")
                nc.vector.reciprocal(r[:], dn[:])
                o1 = wp.tile([P, D1], F32, tag="o1")
                nc.vector.tensor_scalar_mul(o1[:], ps[:, 0:D1], r[:, 0:1])
                nc.vector.tensor_tensor(out=o1[:], in0=o1[:], in1=b2sb[:], op=mybir.AluOpType.add)
                tm = wp.tile([P, D1], F32, tag="tm")
                nc.vector.tensor_scalar_min(tm[:], o1[:], 0.0)
                te = wp.tile([P, D1], F32, tag="te")
                nc.scalar.activation(te[:], tm[:], mybir.ActivationFunctionType.Exp)
                tr = wp.tile([P, D1], F32, tag="tr")
                nc.vector.tensor_scalar_max(tr[:], o1[:], 0.0)
                nc.vector.tensor_tensor(out=te[:], in0=te[:], in1=tr[:], op=mybir.AluOpType.add)
                h3b = wp.tile([P, D1], BF16, tag="h2b")
                nc.vector.tensor_scalar_add(h3b[:], te[:], -1.0)

                # head: logits = h3 @ Wl + bl, then log_softmax
                ps4 = psh.tile([P, CL], F32, tag="hd")
                for kc in range(KCH2):
                    ksz = min(P, D1 - kc * P)
                    tp = pst.tile([P, P], BF16, tag="tp")
                    nc.tensor.transpose(tp[:ksz, :], h3b[:, kc * P : kc * P + ksz], ident[:])
                    h3t = wp.tile([P, P], BF16, tag="h2t")
                    nc.vector.tensor_copy(h3t[:ksz, :], tp[:ksz, :])
                    nc.tensor.matmul(
                        ps4[:], lhsT=h3t[:ksz, :], rhs=wlsb[:ksz, kc * CL : kc * CL + CL],
                        start=(kc == 0), stop=(kc == KCH2 - 1),
                    )
                lg = wp.tile([P, CL], F32, tag="lg")
                nc.vector.tensor_tensor(out=lg[:], in0=ps4[:], in1=blsb[:], op=mybir.AluOpType.add)
                m = wp.tile([P, 1], F32, tag="m")
                nc.vector.reduce_max(m[:], lg[:], axis=mybir.AxisListType.X)
                xs = wp.tile([P, CL], F32, tag="xs")
                nc.vector.tensor_scalar_sub(xs[:], lg[:], m[:, 0:1])
                ex = wp.tile([P, CL], F32, tag="ex")
                sm = wp.tile([P, 1], F32, tag="sm")
                nc.scalar.activation(ex[:], xs[:], mybir.ActivationFunctionType.Exp, accum_out=sm[:])
                ls = wp.tile([P, 1], F32, tag="ls")
                nc.scalar.activation(ls[:], sm[:], mybir.ActivationFunctionType.Ln)
                oo = wp.tile([P, CL], F32, tag="oo")
                nc.vector.tensor_scalar_sub(oo[:], xs[:], ls[:, 0:1])
                nc.sync.dma_start(out_dram[nb * P : (nb + 1) * P, :], oo[:])

    split_multi_waits(nc)
    return nc


# ---------------------------------------------------------------------------
# Host wrapper
# ---------------------------------------------------------------------------
def make_in_maps(cfg, pre, x, W1, att_src1, att_dst1, b1, W2, att_src2, att_dst2,
                 b2, Wl, bl):
    c = derive(cfg)
    NC, NBLK, NDP, KCH1, ND = cfg["NC"], c["NBLK"], c["NDP"], c["KCH1"], c["ND"]
    w1ext, w2ext = fold_weights(cfg, W1, att_src1, att_dst1, W2, att_src2, att_dst2)
    bf = ml_dtypes.bfloat16
    w1b = np.ascontiguousarray(w1ext, dtype=np.float32).astype(bf)
    w2b = np.ascontiguousarray(w2ext, dtype=np.float32).astype(bf)
    wlb = np.ascontiguousarray(Wl, dtype=np.float32).astype(bf)
    b1b = np.tile(np.asarray(b1, np.float32)[None, :], (P, 1))
    b2b = np.tile(np.asarray(b2, np.float32)[None, :], (P, 1))
    blb = np.tile(np.asarray(bl, np.float32)[None, :], (P, 1))

    in_maps = []
    for k in range(NC):
        ids = pre["order_all"][k * ND : (k + 1) * ND]
        xk = np.zeros((NDP, cfg["FIN"]), dtype=np.float32)
        xk[:ND] = np.asarray(x, np.float32)[ids]
        # [NBLK, P(k), KCH, P(n)] layout: row nb*P+kk, col kc*P+nn =
        #   xT[kc*P+kk, nb*P+nn] = xk[nb*P+nn, kc*P+kk]
        xp = np.zeros((NDP, c["FINP"]), dtype=np.float32)
        xp[:, : cfg["FIN"]] = xk
        x4 = xp.reshape(NBLK, P, KCH1, P).transpose(0, 3, 2, 1)
        xt = np.ascontiguousarray(x4.reshape(NBLK * P, KCH1 * P)).astype(bf)
        in_maps.append(
            dict(
                xt=xt,
                idx=np.ascontiguousarray(pre["idx"][k]),
                mask=np.ascontiguousarray(pre["maskb"][k]),
                w1ext=w1b, w2ext=w2b, wl=wlb, b1b=b1b, b2b=b2b, blb=blb,
            )
        )
    return in_maps


def postprocess(cfg, pre, results):
    c = derive(cfg)
    ND, NDP = c["ND"], c["NDP"]
    full = np.concatenate(
        [np.asarray(results[k]["out"])[:ND] for k in range(cfg["NC"])], axis=0
    )
    out = np.empty((cfg["N"], cfg["C"]), dtype=np.float32)
    out[pre["order_all"].reshape(cfg["NC"], ND).reshape(-1)] = full
    return out


def kernel(**inputs):
    from concourse.bass_utils import run_bass_kernel_spmd

    cfg = full_cfg()
    pre = preprocess(cfg, np.asarray(inputs["edge_index"]))
    nc = build_program(cfg, pre["cblk"], pre["CT"], pre["off"])
    in_maps = make_in_maps(
        cfg, pre, inputs["x"],
        inputs["W1"], inputs["att_src1"], inputs["att_dst1"], inputs["b1"],
        inputs["W2"], inputs["att_src2"], inputs["att_dst2"], inputs["b2"],
        inputs["Wl"], inputs["bl"],
    )
    res = run_bass_kernel_spmd(nc, in_maps, list(range(cfg["NC"])))
    return postprocess(cfg, pre, res.results)
